# revision 5
# baseline (speedup 1.0000x reference)
"""GNN message-passing (segment-mean + linear + relu) Trainium2 kernel.

Sharding: the batch's unique seed nodes are partitioned across 8 cores
round-robin over the sorted unique-node list (so each core's node ids span
all int16 index chunks evenly); edges are colocated with their source
node's core, and only edges whose source is a seed node are kept (the rest
cannot affect the output). Features are replicated in HBM on every core
(f16 copy only).

Per-core device algorithm (one fused phase-1 stream):
  Each 128-slot block's entry list = its kept edges PLUS one self-entry per
  real slot. For each (4-block group, 32768-node chunk) the entries' dst
  features are fetched with dma_gather (int16 indices, pieces sized to the
  SWDGE descriptor ring), a [entry, 256] selector S is built on the DVE with
  one batched is_equal against an iota-256 row (entry seg value = slot for
  edges, 128+slot for self-entries, -1 for padding), and the PE accumulates
  G^T @ S into PSUM [feat, 256] per block: left 128 columns = neighbor sum,
  right 128 = self features.
  Phase 2 per block: DVE multiplies the neighbor half by the per-slot
  inverse degree (PSUM -> SBUF), ACT copies the self half, two matmuls
  (lhsT=W2^T, lhsT=W1^T) accumulate into one PSUM [dout, slot] tile, ACT
  applies relu, DMA out.

Output: [D, U_cap] per core (column s = output row for that core's slot s);
the host scatters columns back to the [50000, 128] batch.
"""

import sys

for _p in ("/opt/trn_rl_repo",):
    if _p not in sys.path:
        sys.path.insert(0, _p)

import numpy as np

import concourse.bacc as bacc
import concourse.bass as bass
import concourse.mybir as mybir
from concourse.library_config import mlp
from concourse.tile import TileContext

P = 128
CHUNK = 32768  # int16 index range for dma_gather


def _roundup(x, m):
    return (x + m - 1) // m * m


def _wrap16(vals, dtype=np.int16):
    """dma_gather index layout: wrapped[p, s] = vals[s*16 + (p % 16)],
    replicated across all 128 partitions."""
    vals = np.asarray(vals)
    n = vals.shape[0]
    assert n % 16 == 0
    w = vals.reshape(n // 16, 16).T.astype(dtype)  # [16, n/16]
    return np.tile(w, (8, 1))  # [128, n/16]


def preprocess(nodes, features, edge_index, W, b, n_cores=8, nbg_blocks=4):
    """Host-side index-space preprocessing. Returns (plan, in_maps, assemble)
    where assemble(core_outputs) -> full [B, D] output."""
    nodes = np.asarray(nodes).astype(np.int64)
    src = np.asarray(edge_index[0]).astype(np.int64)
    dst = np.asarray(edge_index[1]).astype(np.int64)
    features = np.asarray(features, dtype=np.float32)
    W = np.asarray(W, dtype=np.float32)
    b = np.asarray(b, dtype=np.float32)

    N, D = features.shape
    assert D == 128 and W.shape == (D, 2 * D)
    nchunk = (N + CHUNK - 1) // CHUNK

    features_h = np.ascontiguousarray(features.astype(np.float16))
    uniq, inv = np.unique(nodes, return_inverse=True)
    U = len(uniq)
    deg = np.bincount(src, minlength=N).astype(np.float64)

    # interleave unique nodes across cores so every core's node-id set spans
    # all int16 chunks roughly evenly (uniq is sorted: a contiguous slice
    # would concentrate in one chunk)
    core_of = np.arange(U) % n_cores
    core_idx = [np.arange(c, U, n_cores) for c in range(n_cores)]
    pos = np.zeros(U, dtype=np.int64)
    chunk_counts = np.zeros((n_cores, nchunk), dtype=np.int64)
    rank_in_run = [None] * n_cores
    chunk_of = [None] * n_cores
    for c in range(n_cores):
        nds = uniq[core_idx[c]]
        ch = nds // CHUNK  # non-decreasing
        run_starts = np.searchsorted(ch, np.arange(nchunk))
        chunk_counts[c] = np.searchsorted(ch, np.arange(nchunk) + 1) - run_starts
        rank_in_run[c] = np.arange(len(nds)) - run_starts[ch]
        chunk_of[c] = ch
    CSELF_CAP = max(128, _roundup(int(chunk_counts.max()), P))
    U_cap = nchunk * CSELF_CAP
    NBLK = U_cap // P
    NBLK_pad = _roundup(NBLK, nbg_blocks)
    NBG = NBLK_pad // nbg_blocks

    for c in range(n_cores):
        pos[core_idx[c]] = rank_in_run[c] + chunk_of[c] * CSELF_CAP

    # per-core slot tables
    slot_node = np.zeros((n_cores, U_cap), dtype=np.int64)
    slot_real = np.zeros((n_cores, U_cap), dtype=bool)
    slot_invdeg = np.zeros((n_cores, U_cap), dtype=np.float32)
    for c in range(n_cores):
        ci = core_idx[c]
        slot_node[c, pos[ci]] = uniq[ci]
        slot_real[c, pos[ci]] = True
        slot_invdeg[c, pos[ci]] = (1.0 / np.maximum(deg[uniq[ci]], 1.0)).astype(
            np.float32
        )

    # edges: keep only those whose src is a seed node
    upos_of_node = np.full(N, -1, dtype=np.int64)
    upos_of_node[uniq] = np.arange(U)
    eu = upos_of_node[src]
    keep = eu >= 0
    eu = eu[keep]
    ed = dst[keep]
    ecore = core_of[eu]
    epos = pos[eu]

    # append one self-entry per real slot: dst = the slot's own node, seg
    # value offset by +128 so the iota-256 one-hot routes it to the self
    # half of the PSUM tile
    self_core = []
    self_pos = []
    self_dst = []
    for c in range(n_cores):
        rp = np.flatnonzero(slot_real[c])
        self_core.append(np.full(len(rp), c, dtype=np.int64))
        self_pos.append(rp)
        self_dst.append(slot_node[c, rp])
    ecore = np.concatenate([ecore] + self_core)
    epos = np.concatenate([epos] + self_pos)
    ed = np.concatenate([ed] + self_dst)
    is_self = np.zeros(len(ecore), dtype=bool)
    is_self[-sum(len(x) for x in self_pos):] = True

    eblock = epos // P
    echunk = ed // CHUNK

    # per (core, block, chunk) counts -> shared tile counts T[b, k]
    flat = (ecore * NBLK_pad + eblock) * nchunk + echunk
    cnt = np.bincount(flat, minlength=n_cores * NBLK_pad * nchunk).reshape(
        n_cores, NBLK_pad, nchunk
    )
    T = np.ceil(cnt.max(axis=0) / P).astype(np.int64)  # [NBLK_pad, nchunk]
    # blocks that are padding on EVERY core: no core has a real slot there
    real_csb = np.maximum(
        1, -(-chunk_counts.max(axis=0) // P)
    )  # [nchunk] blocks actually used per chunk run
    CSB_ = CSELF_CAP // P
    skip_blocks = set()
    for k in range(nchunk):
        for bloc in range(int(real_csb[k]), CSB_):
            skip_blocks.add(k * CSB_ + bloc)
    for blk in range(NBLK, NBLK_pad):
        skip_blocks.add(blk)
    T[sorted(skip_blocks), :] = 0

    # per-gather capacities and offsets
    EG_CAP = np.zeros((NBG, nchunk), dtype=np.int64)  # num_idxs per gather
    for g in range(NBG):
        for k in range(nchunk):
            EG_CAP[g, k] = P * int(T[g * nbg_blocks : (g + 1) * nbg_blocks, k].sum())
    idx_off = np.zeros((NBG, nchunk), dtype=np.int64)  # offset into idx array /16
    acc = 0
    for g in range(NBG):
        for k in range(nchunk):
            idx_off[g, k] = acc
            acc += EG_CAP[g, k] // 16
    IDX_COLS = int(acc)

    # seg columns: global tile order (g, k, b, t)
    col0 = np.zeros((NBG, nchunk), dtype=np.int64)
    acc = 0
    for g in range(NBG):
        for k in range(nchunk):
            col0[g, k] = acc
            acc += EG_CAP[g, k] // P
    T_TOTAL = int(acc)

    # build per-core arrays
    in_maps = []
    for c in range(n_cores):
        m = ecore == c
        ceb, cek, ced, cep = eblock[m], echunk[m], ed[m], epos[m]
        csf = is_self[m]
        order = np.lexsort((ced, cek, ceb))
        ceb, cek, ced, cep, csf = (
            ceb[order], cek[order], ced[order], cep[order], csf[order]
        )
        # group boundaries per (block, chunk)
        key = ceb * nchunk + cek
        bc_cnt = np.bincount(key, minlength=NBLK_pad * nchunk).reshape(
            NBLK_pad, nchunk
        )
        flatc = np.concatenate([[0], np.cumsum(bc_cnt.reshape(-1))[:-1]])
        starts = flatc.reshape(NBLK_pad, nchunk)

        edge_idx_vals = np.zeros(IDX_COLS * 16, dtype=np.int64)
        seg = np.full((P, T_TOTAL), -1.0, dtype=np.float16)
        for g in range(NBG):
            for k in range(nchunk):
                base_i = idx_off[g, k] * 16
                base_t = col0[g, k]
                off = 0
                for bb in range(nbg_blocks):
                    blk = g * nbg_blocks + bb
                    tcount = int(T[blk, k])
                    if tcount == 0:
                        continue
                    n = int(bc_cnt[blk, k])
                    s0 = int(starts[blk, k])
                    # idx values: dst - k*CHUNK (pad -> 0)
                    vals = np.zeros(tcount * P, dtype=np.int64)
                    vals[:n] = ced[s0 : s0 + n] - k * CHUNK
                    edge_idx_vals[base_i + off * P : base_i + (off + tcount) * P] = (
                        vals
                    )
                    # seg values: slot within block (+128 for self; pad -> -1)
                    sv = np.full(tcount * P, -1.0, dtype=np.float16)
                    sv[:n] = (
                        cep[s0 : s0 + n] - blk * P + 128 * csf[s0 : s0 + n]
                    ).astype(np.float32)
                    seg[:, base_t + off : base_t + off + tcount] = sv.reshape(
                        tcount, P
                    ).T
                    off += tcount
        assert np.all(edge_idx_vals >= 0) and np.all(edge_idx_vals < CHUNK)

        # inverse-degree per (block, slot), replicated across partitions
        invw = np.tile(
            slot_invdeg[c].reshape(1, NBLK, P).astype(np.float32), (P, 1, 1)
        )
        if NBLK_pad > NBLK:
            invw = np.concatenate(
                [invw, np.zeros((P, NBLK_pad - NBLK, P), np.float32)], axis=1
            )

        in_maps.append(
            {
                "features_h": features_h,
                "edge_idx": _wrap16(edge_idx_vals),
                "seg": seg,
                "invdeg": invw.reshape(P, NBLK_pad * P),
                "w1t": W[:, :D].T.copy(),
                "w2t": W[:, D:].T.copy(),
                "bias_col": np.tile(b.reshape(D, 1), (1, P)),
                "iota": np.tile(np.arange(256, dtype=np.float16), (P, 1)),
            }
        )

    plan = {
        "N": N,
        "D": D,
        "nchunk": nchunk,
        "CSELF_CAP": CSELF_CAP,
        "U_cap": U_cap,
        "NBLK": NBLK,
        "NBLK_pad": NBLK_pad,
        "NBG": NBG,
        "nbg_blocks": nbg_blocks,
        "T": T,
        "EG_CAP": EG_CAP,
        "idx_off": idx_off,
        "col0": col0,
        "IDX_COLS": IDX_COLS,
        "T_TOTAL": T_TOTAL,
        "n_cores": n_cores,
        "bias_nonzero": bool(np.any(b != 0)),
        "skip_blocks": skip_blocks,
        "real_csb": real_csb,
    }

    out_core = core_of[inv]
    out_pos = pos[inv]

    def assemble(core_outputs):
        stacked = np.stack(core_outputs)  # [n_cores, D, U_cap_pad]
        return np.ascontiguousarray(
            stacked[out_core, :, out_pos]
        )

    return plan, in_maps, assemble


def build_kernel(plan, reps=1, ge_bufs=8, s_bufs=4, blk_bufs=8, p1_bufs=2,
                 p2_bufs=2, ni_tiles=8, scratch=32768, nq=4):
    N, D = plan["N"], plan["D"]
    nchunk = plan["nchunk"]
    NBLK_pad = plan["NBLK_pad"]
    NBG = plan["NBG"]
    nbg_blocks = plan["nbg_blocks"]
    T = plan["T"]
    EG_CAP = plan["EG_CAP"]
    idx_off = plan["idx_off"]
    col0 = plan["col0"]
    IDX_COLS = plan["IDX_COLS"]
    T_TOTAL = plan["T_TOTAL"]
    EG_TILES_MAX = int(EG_CAP.max()) // P

    f32 = mybir.dt.float32
    f16 = mybir.dt.float16
    NQ = nq  # SWDGE queues, round-robin
    NI_TILES = ni_tiles  # gather piece size (tiles of 128 idxs)
    nc = bacc.Bacc(
        "TRN2",
        target_bir_lowering=False,
        num_swdge_queues=NQ,
        dynamic_dma_scratch_size=scratch,
    )
    # one reusable Pool-engine register for dma_gather valid-index counts
    cnt_reg = list(
        nc.alloc_registers("gather_cnt", engines=[mybir.EngineType.Pool])
    )[0]
    qrr = [0]
    cnt_cache = [None]

    def emit_gather(out3d, tile0, ntiles, in_ap, idx_tile, idxcol0):
        """dma_gather split into <=NI_TILES*128-index pieces."""
        for p0 in range(0, ntiles, NI_TILES):
            p1 = min(p0 + NI_TILES, ntiles)
            ni = (p1 - p0) * P
            if cnt_cache[0] != ni:
                nc.gpsimd.reg_mov(cnt_reg, ni)
                cnt_cache[0] = ni
            nc.gpsimd.dma_gather(
                out_ap=out3d[:, tile0 + p0 : tile0 + p1, :],
                in_ap=in_ap,
                idxs_ap=idx_tile[:, idxcol0 + p0 * 8 : idxcol0 + p1 * 8],
                num_idxs=ni,
                num_idxs_reg=cnt_reg,
                elem_size=D,
                queue_num=qrr[0] % NQ,
            )
            qrr[0] += 1

    feat_h = nc.dram_tensor("features_h", [N, D], f16, kind="ExternalInput")
    edge_idx_d = nc.dram_tensor(
        "edge_idx", [P, IDX_COLS], mybir.dt.int16, kind="ExternalInput"
    )
    seg_d = nc.dram_tensor("seg", [P, T_TOTAL], f16, kind="ExternalInput")
    invdeg_d = nc.dram_tensor(
        "invdeg", [P, NBLK_pad * P], f32, kind="ExternalInput"
    )
    w1t_d = nc.dram_tensor("w1t", [D, D], f32, kind="ExternalInput")
    w2t_d = nc.dram_tensor("w2t", [D, D], f32, kind="ExternalInput")
    bias_d = nc.dram_tensor("bias_col", [D, P], f32, kind="ExternalInput")
    iota_d = nc.dram_tensor("iota", [P, 256], f16, kind="ExternalInput")
    out_d = nc.dram_tensor(
        "out", [D, NBLK_pad * P], f32, kind="ExternalOutput"
    )

    with TileContext(nc) as tc:
        with (
            tc.tile_pool(name="const", bufs=1) as const_pool,
            tc.tile_pool(name="ge", bufs=ge_bufs) as ge_pool,
            tc.tile_pool(name="s", bufs=s_bufs) as s_pool,
            tc.tile_pool(name="blk", bufs=blk_bufs) as blk_pool,
            tc.tile_pool(name="psum1", bufs=p1_bufs, space="PSUM") as psum1_pool,
            tc.tile_pool(name="psum2", bufs=p2_bufs, space="PSUM") as psum2_pool,
        ):
            nc.gpsimd.load_library(mlp)

            def load_const(dram, shape, dtype=f32, tag=None):
                t = const_pool.tile(shape, dtype, tag=tag)
                nc.sync.dma_start(t[:], dram[:])
                return t

            edge_idx = load_const(
                edge_idx_d, [P, IDX_COLS], mybir.dt.int16, tag="edge_idx"
            )
            seg = load_const(seg_d, [P, T_TOTAL], f16, tag="seg")
            invdeg = load_const(
                invdeg_d, [P, NBLK_pad * P], tag="invdeg"
            ).rearrange("p (b s) -> p b s", s=P)
            w1t = load_const(w1t_d, [D, D], tag="w1t")
            w2t = load_const(w2t_d, [D, D], tag="w2t")
            bias_col = load_const(bias_d, [D, P], tag="bias_col")
            iota = load_const(iota_d, [P, 256], f16, tag="iota")

            for _rep in range(reps):
                for g in range(NBG):
                    gbuf = {}
                    stile = {}
                    for k in range(nchunk):
                        tgk = int(EG_CAP[g, k]) // P
                        if tgk == 0:
                            continue
                        gb = ge_pool.tile([P, EG_TILES_MAX, D], f16, tag="ge")
                        emit_gather(
                            gb, 0, tgk,
                            feat_h[k * CHUNK :, :],
                            edge_idx, int(idx_off[g, k]),
                        )
                        gbuf[k] = gb
                        # batched one-hot: S[p, t, w] = (seg[p, col0+t] == iota[w])
                        # w in [0, 256): left half neighbor edges, right half self
                        st = s_pool.tile([P, EG_TILES_MAX, 256], f16, tag="s")
                        c0 = int(col0[g, k])
                        seg_rep = seg[:, c0 : c0 + tgk].rearrange(
                            "p (t o) -> p t o", o=1
                        ).to_broadcast([P, tgk, 256])
                        iota_rep = iota[:, :].rearrange(
                            "p (o w) -> p o w", o=1
                        ).to_broadcast([P, tgk, 256])
                        nc.vector.tensor_tensor(
                            out=st[:, :tgk, :],
                            in0=seg_rep,
                            in1=iota_rep,
                            op=mybir.AluOpType.is_equal,
                        )
                        stile[k] = st

                    psum1 = psum1_pool.tile([P, nbg_blocks, 256], f32, tag="p1")
                    # per-block static schedule of (chunk, local tile) pairs
                    sched = [[] for _ in range(nbg_blocks)]
                    for k in range(nchunk):
                        off = 0
                        for bb in range(nbg_blocks):
                            tcount = int(T[g * nbg_blocks + bb, k])
                            for t in range(tcount):
                                sched[bb].append((k, off + t))
                            off += tcount
                    # block-major: one PSUM accumulation group open at a time
                    for bb in range(nbg_blocks):
                        total = len(sched[bb])
                        for i, (k, t) in enumerate(sched[bb]):
                            nc.tensor.matmul(
                                out=psum1[:, bb, :],
                                lhsT=gbuf[k][:, t, :],
                                rhs=stile[k][:, t, :],
                                start=(i == 0),
                                stop=(i == total - 1),
                            )

                    for bb in range(nbg_blocks):
                        blk = g * nbg_blocks + bb
                        if blk in plan["skip_blocks"]:
                            continue
                        # neighbor mean: PSUM left half * invdeg -> SBUF
                        msum = blk_pool.tile([P, P], f32, tag="msum")
                        nc.vector.tensor_tensor(
                            out=msum[:],
                            in0=psum1[:, bb, 0:128],
                            in1=invdeg[:, blk, :],
                            op=mybir.AluOpType.mult,
                        )
                        # self features: PSUM right half -> SBUF
                        selfT = blk_pool.tile([P, P], f32, tag="selfT")
                        nc.scalar.activation(
                            selfT[:],
                            psum1[:, bb, 128:256],
                            mybir.ActivationFunctionType.Copy,
                        )
                        psum_o = psum2_pool.tile([P, P], f32, tag="po")
                        nc.tensor.matmul(
                            out=psum_o[:], lhsT=w2t[:], rhs=msum[:],
                            start=True, stop=False,
                        )
                        nc.tensor.matmul(
                            out=psum_o[:], lhsT=w1t[:], rhs=selfT[:],
                            start=False, stop=True,
                        )
                        out_sb = blk_pool.tile([P, P], f32, tag="osb")
                        if plan["bias_nonzero"]:
                            o1 = blk_pool.tile([P, P], f32, tag="o1")
                            nc.vector.tensor_tensor(
                                out=o1[:], in0=psum_o[:], in1=bias_col[:],
                                op=mybir.AluOpType.add,
                            )
                            nc.scalar.activation(
                                out_sb[:], o1[:], mybir.ActivationFunctionType.Relu
                            )
                        else:
                            nc.scalar.activation(
                                out_sb[:], psum_o[:],
                                mybir.ActivationFunctionType.Relu,
                            )
                        nc.sync.dma_start(
                            out_d[:, blk * P : (blk + 1) * P], out_sb[:]
                        )

    nc.compile()
    return nc


_RUN_KWARGS = {}


def run_on_hw(nc, in_maps, n_cores, **kwargs):
    from concourse.bass_utils import run_bass_kernel_spmd

    return run_bass_kernel_spmd(nc, in_maps, list(range(n_cores)), **kwargs)


def kernel(nodes, features, edge_index, W, b):
    """Full-input entry point: shards internally across 8 NeuronCores."""
    n_cores = 8
    plan, in_maps, assemble = preprocess(
        nodes, features, edge_index, W, b, n_cores=n_cores
    )
    nc = build_kernel(plan)
    res = run_on_hw(nc, in_maps, n_cores, **_RUN_KWARGS)
    outs = [np.asarray(r["out"]) for r in res.results]
    return np.ascontiguousarray(assemble(outs).astype(np.float32))


# revision 10
# speedup vs baseline: 1.4257x; 1.4257x over previous
"""GNN message-passing (segment-mean + linear + relu) Trainium2 kernel.

Sharding: the batch's unique seed nodes are partitioned across 8 cores
round-robin over the sorted unique-node list (so each core's node ids span
all int16 index chunks evenly); edges are colocated with their source
node's core, and only edges whose source is a seed node are kept (the rest
cannot affect the output). Features are replicated in HBM on every core
(f16 copy only).

Per-core device algorithm (one fused phase-1 stream):
  Each 128-slot block's entry list = its kept edges PLUS one self-entry per
  real slot. For each (4-block group, 32768-node chunk) the entries' dst
  features are fetched with dma_gather (int16 indices, pieces sized to the
  SWDGE descriptor ring), a [entry, 256] selector S is built on the DVE with
  one batched is_equal against an iota-256 row (entry seg value = slot for
  edges, 128+slot for self-entries, -1 for padding), and the PE accumulates
  G^T @ S into PSUM [feat, 256] per block: left 128 columns = neighbor sum,
  right 128 = self features.
  Phase 2 per block: DVE multiplies the neighbor half by the per-slot
  inverse degree (PSUM -> SBUF), ACT copies the self half, two matmuls
  (lhsT=W2^T, lhsT=W1^T) accumulate into one PSUM [dout, slot] tile, ACT
  applies relu, DMA out.

Output: [D, U_cap] per core (column s = output row for that core's slot s);
the host scatters columns back to the [50000, 128] batch.
"""

import sys

for _p in ("/opt/trn_rl_repo",):
    if _p not in sys.path:
        sys.path.insert(0, _p)

import numpy as np

import concourse.bacc as bacc
import concourse.bass as bass
import concourse.mybir as mybir
from concourse.library_config import mlp
from concourse.tile import TileContext

P = 128
CHUNK = 32768  # int16 index range for dma_gather


def _roundup(x, m):
    return (x + m - 1) // m * m


def _wrap16(vals, dtype=np.int16):
    """dma_gather index layout: wrapped[p, s] = vals[s*16 + (p % 16)],
    replicated across all 128 partitions."""
    vals = np.asarray(vals)
    n = vals.shape[0]
    assert n % 16 == 0
    w = vals.reshape(n // 16, 16).T.astype(dtype)  # [16, n/16]
    return np.tile(w, (8, 1))  # [128, n/16]


def preprocess(nodes, features, edge_index, W, b, n_cores=8, nbg_blocks=4):
    """Host-side index-space preprocessing. Returns (plan, in_maps, assemble)
    where assemble(core_outputs) -> full [B, D] output."""
    nodes = np.asarray(nodes).astype(np.int64)
    src = np.asarray(edge_index[0]).astype(np.int64)
    dst = np.asarray(edge_index[1]).astype(np.int64)
    features = np.asarray(features, dtype=np.float32)
    W = np.asarray(W, dtype=np.float32)
    b = np.asarray(b, dtype=np.float32)

    N, D = features.shape
    assert D == 128 and W.shape == (D, 2 * D)
    nchunk = (N + CHUNK - 1) // CHUNK

    features_h = np.ascontiguousarray(features.astype(np.float16))
    uniq, inv = np.unique(nodes, return_inverse=True)
    U = len(uniq)
    deg = np.bincount(src, minlength=N).astype(np.float64)

    # interleave unique nodes across cores so every core's node-id set spans
    # all int16 chunks roughly evenly (uniq is sorted: a contiguous slice
    # would concentrate in one chunk)
    core_of = np.arange(U) % n_cores
    core_idx = [np.arange(c, U, n_cores) for c in range(n_cores)]
    pos = np.zeros(U, dtype=np.int64)
    chunk_counts = np.zeros((n_cores, nchunk), dtype=np.int64)
    rank_in_run = [None] * n_cores
    chunk_of = [None] * n_cores
    for c in range(n_cores):
        nds = uniq[core_idx[c]]
        ch = nds // CHUNK  # non-decreasing
        run_starts = np.searchsorted(ch, np.arange(nchunk))
        chunk_counts[c] = np.searchsorted(ch, np.arange(nchunk) + 1) - run_starts
        rank_in_run[c] = np.arange(len(nds)) - run_starts[ch]
        chunk_of[c] = ch
    CSELF_CAP = max(128, _roundup(int(chunk_counts.max()), P))
    U_cap = nchunk * CSELF_CAP
    NBLK = U_cap // P
    NBLK_pad = _roundup(NBLK, nbg_blocks)
    NBG = NBLK_pad // nbg_blocks

    for c in range(n_cores):
        pos[core_idx[c]] = rank_in_run[c] + chunk_of[c] * CSELF_CAP

    # per-core slot tables
    slot_node = np.zeros((n_cores, U_cap), dtype=np.int64)
    slot_real = np.zeros((n_cores, U_cap), dtype=bool)
    slot_invdeg = np.zeros((n_cores, U_cap), dtype=np.float32)
    for c in range(n_cores):
        ci = core_idx[c]
        slot_node[c, pos[ci]] = uniq[ci]
        slot_real[c, pos[ci]] = True
        slot_invdeg[c, pos[ci]] = (1.0 / np.maximum(deg[uniq[ci]], 1.0)).astype(
            np.float32
        )

    # edges: keep only those whose src is a seed node
    upos_of_node = np.full(N, -1, dtype=np.int64)
    upos_of_node[uniq] = np.arange(U)
    eu = upos_of_node[src]
    keep = eu >= 0
    eu = eu[keep]
    ed = dst[keep]
    ecore = core_of[eu]
    epos = pos[eu]

    # append one self-entry per real slot: dst = the slot's own node, seg
    # value offset by +128 so the iota-256 one-hot routes it to the self
    # half of the PSUM tile
    self_core = []
    self_pos = []
    self_dst = []
    for c in range(n_cores):
        rp = np.flatnonzero(slot_real[c])
        self_core.append(np.full(len(rp), c, dtype=np.int64))
        self_pos.append(rp)
        self_dst.append(slot_node[c, rp])
    ecore = np.concatenate([ecore] + self_core)
    epos = np.concatenate([epos] + self_pos)
    ed = np.concatenate([ed] + self_dst)
    is_self = np.zeros(len(ecore), dtype=bool)
    is_self[-sum(len(x) for x in self_pos):] = True

    eblock = epos // P

    # Overlapping gather windows: 4 windows of 32768 rows cover N=100000 with
    # 31072 rows of total overlap. Entries whose dst falls in an overlap
    # region can be served by either adjacent window; a cascading flex-fill
    # packs every window to its cross-core ceil so the shared tile schedule
    # approaches ceil(total/128) per block instead of one ceil per window.
    B4 = np.array([0, 22500, 45000, 67232], dtype=np.int64)
    assert nchunk == 4 and N <= B4[3] + CHUNK
    REDGE = np.array(
        [B4[1], B4[0] + CHUNK, B4[2], B4[1] + CHUNK, B4[3], B4[2] + CHUNK, N],
        dtype=np.int64,
    )  # region r: even = forced window r//2, odd = flex (r//2 or r//2+1)
    ereg = np.searchsorted(REDGE, ed, side="right")  # [0..6]

    # per (core, block, region) counts
    flatr = (ecore * NBLK_pad + eblock) * 7 + ereg
    rc = np.bincount(flatr, minlength=n_cores * NBLK_pad * 7).reshape(
        n_cores, NBLK_pad, 7
    )
    T = np.zeros((NBLK_pad, nchunk), dtype=np.int64)
    xq = np.zeros((n_cores, NBLK_pad, 3), dtype=np.int64)  # flex taken left
    need = rc[:, :, 0].astype(np.int64)
    for k in range(nchunk):
        T[:, k] = -(-need.max(axis=0) // P)
        if k < 3:
            flex = rc[:, :, 2 * k + 1]
            xq[:, :, k] = np.clip(P * T[None, :, k] - need, 0, flex)
            need = rc[:, :, 2 * k + 2] + (flex - xq[:, :, k])
    cnt = None  # superseded by the window machinery above
    # blocks that are padding on EVERY core: no core has a real slot there
    real_csb = np.maximum(
        1, -(-chunk_counts.max(axis=0) // P)
    )  # [nchunk] blocks actually used per chunk run
    CSB_ = CSELF_CAP // P
    skip_blocks = set()
    for k in range(nchunk):
        for bloc in range(int(real_csb[k]), CSB_):
            skip_blocks.add(k * CSB_ + bloc)
    for blk in range(NBLK, NBLK_pad):
        skip_blocks.add(blk)
    T[sorted(skip_blocks), :] = 0

    # per-gather capacities and offsets
    EG_CAP = np.zeros((NBG, nchunk), dtype=np.int64)  # num_idxs per gather
    for g in range(NBG):
        for k in range(nchunk):
            EG_CAP[g, k] = P * int(T[g * nbg_blocks : (g + 1) * nbg_blocks, k].sum())
    idx_off = np.zeros((NBG, nchunk), dtype=np.int64)  # offset into idx array /16
    acc = 0
    for g in range(NBG):
        for k in range(nchunk):
            idx_off[g, k] = acc
            acc += EG_CAP[g, k] // 16
    IDX_COLS = int(acc)

    # seg columns: global tile order (g, k, b, t)
    col0 = np.zeros((NBG, nchunk), dtype=np.int64)
    acc = 0
    for g in range(NBG):
        for k in range(nchunk):
            col0[g, k] = acc
            acc += EG_CAP[g, k] // P
    T_TOTAL = int(acc)

    # build per-core arrays
    in_maps = []
    for c in range(n_cores):
        m = ecore == c
        ceb, ced, cep = eblock[m], ed[m], epos[m]
        creg = ereg[m]
        csf = is_self[m]
        # window assignment: forced for even regions; flex regions send the
        # first xq (in dst order within the (block, region) group) left
        o2 = np.lexsort((ced, creg, ceb))
        gkey = ceb[o2] * 7 + creg[o2]
        change = np.concatenate([[True], gkey[1:] != gkey[:-1]])
        gstart = np.maximum.accumulate(
            np.where(change, np.arange(len(gkey)), 0)
        )
        rank = np.arange(len(gkey)) - gstart
        win_s = creg[o2] // 2
        isflex = (creg[o2] % 2) == 1
        take_left = rank < xq[c, ceb[o2], np.minimum(win_s, 2)]
        win_s = np.where(isflex & ~take_left, win_s + 1, win_s)
        cek = np.empty(len(win_s), dtype=np.int64)
        cek[o2] = win_s
        order = np.lexsort((ced, cek, ceb))
        ceb, cek, ced, cep, csf = (
            ceb[order], cek[order], ced[order], cep[order], csf[order]
        )
        # group boundaries per (block, chunk)
        key = ceb * nchunk + cek
        bc_cnt = np.bincount(key, minlength=NBLK_pad * nchunk).reshape(
            NBLK_pad, nchunk
        )
        flatc = np.concatenate([[0], np.cumsum(bc_cnt.reshape(-1))[:-1]])
        starts = flatc.reshape(NBLK_pad, nchunk)

        edge_idx_vals = np.zeros(IDX_COLS * 16, dtype=np.int64)
        seg = np.full((P, T_TOTAL), -1.0, dtype=np.float16)
        for g in range(NBG):
            for k in range(nchunk):
                base_i = idx_off[g, k] * 16
                base_t = col0[g, k]
                off = 0
                for bb in range(nbg_blocks):
                    blk = g * nbg_blocks + bb
                    tcount = int(T[blk, k])
                    if tcount == 0:
                        continue
                    n = int(bc_cnt[blk, k])
                    s0 = int(starts[blk, k])
                    # idx values: dst - window_base (pad -> 0)
                    vals = np.zeros(tcount * P, dtype=np.int64)
                    vals[:n] = ced[s0 : s0 + n] - B4[k]
                    edge_idx_vals[base_i + off * P : base_i + (off + tcount) * P] = (
                        vals
                    )
                    # seg values: slot within block (+128 for self; pad -> -1)
                    sv = np.full(tcount * P, -1.0, dtype=np.float16)
                    sv[:n] = (
                        cep[s0 : s0 + n] - blk * P + 128 * csf[s0 : s0 + n]
                    ).astype(np.float32)
                    seg[:, base_t + off : base_t + off + tcount] = sv.reshape(
                        tcount, P
                    ).T
                    off += tcount
        assert np.all(edge_idx_vals >= 0) and np.all(edge_idx_vals < CHUNK)

        # inverse-degree per (block, slot), replicated across partitions
        invw = np.tile(
            slot_invdeg[c].reshape(1, NBLK, P).astype(np.float32), (P, 1, 1)
        )
        if NBLK_pad > NBLK:
            invw = np.concatenate(
                [invw, np.zeros((P, NBLK_pad - NBLK, P), np.float32)], axis=1
            )

        in_maps.append(
            {
                "features_h": features_h,
                "edge_idx": _wrap16(edge_idx_vals),
                "seg": seg,
                "invdeg": invw.reshape(P, NBLK_pad * P),
                "w1t": W[:, :D].T.copy(),
                "w2t": W[:, D:].T.copy(),
                "bias_col": np.tile(b.reshape(D, 1), (1, P)),
                "iota": np.tile(np.arange(256, dtype=np.float16), (P, 1)),
            }
        )

    plan = {
        "N": N,
        "D": D,
        "nchunk": nchunk,
        "CSELF_CAP": CSELF_CAP,
        "U_cap": U_cap,
        "NBLK": NBLK,
        "NBLK_pad": NBLK_pad,
        "NBG": NBG,
        "nbg_blocks": nbg_blocks,
        "T": T,
        "EG_CAP": EG_CAP,
        "idx_off": idx_off,
        "col0": col0,
        "IDX_COLS": IDX_COLS,
        "T_TOTAL": T_TOTAL,
        "n_cores": n_cores,
        "bias_nonzero": bool(np.any(b != 0)),
        "skip_blocks": skip_blocks,
        "real_csb": real_csb,
        "win_base": [int(x) for x in B4],
    }

    out_core = core_of[inv]
    out_pos = pos[inv]

    def assemble(core_outputs):
        stacked = np.stack(core_outputs)  # [n_cores, D, U_cap_pad]
        return np.ascontiguousarray(
            stacked[out_core, :, out_pos]
        )

    return plan, in_maps, assemble


def build_kernel(plan, reps=1, ge_bufs=8, s_bufs=4, blk_bufs=8, p1_bufs=2,
                 p2_bufs=2, ni_tiles=8, scratch=32768, nq=4):
    N, D = plan["N"], plan["D"]
    nchunk = plan["nchunk"]
    NBLK_pad = plan["NBLK_pad"]
    NBG = plan["NBG"]
    nbg_blocks = plan["nbg_blocks"]
    T = plan["T"]
    EG_CAP = plan["EG_CAP"]
    idx_off = plan["idx_off"]
    col0 = plan["col0"]
    IDX_COLS = plan["IDX_COLS"]
    T_TOTAL = plan["T_TOTAL"]
    EG_TILES_MAX = int(EG_CAP.max()) // P

    f32 = mybir.dt.float32
    f16 = mybir.dt.float16
    NQ = nq  # SWDGE queues, round-robin
    NI_TILES = ni_tiles  # gather piece size (tiles of 128 idxs)
    nc = bacc.Bacc(
        "TRN2",
        target_bir_lowering=False,
        num_swdge_queues=NQ,
        dynamic_dma_scratch_size=scratch,
    )
    # one reusable Pool-engine register for dma_gather valid-index counts
    cnt_reg = list(
        nc.alloc_registers("gather_cnt", engines=[mybir.EngineType.Pool])
    )[0]
    qrr = [0]
    cnt_cache = [None]

    def emit_gather(out3d, tile0, ntiles, in_ap, idx_tile, idxcol0):
        """dma_gather split into <=NI_TILES*128-index pieces."""
        for p0 in range(0, ntiles, NI_TILES):
            p1 = min(p0 + NI_TILES, ntiles)
            ni = (p1 - p0) * P
            if cnt_cache[0] != ni:
                nc.gpsimd.reg_mov(cnt_reg, ni)
                cnt_cache[0] = ni
            nc.gpsimd.dma_gather(
                out_ap=out3d[:, tile0 + p0 : tile0 + p1, :],
                in_ap=in_ap,
                idxs_ap=idx_tile[:, idxcol0 + p0 * 8 : idxcol0 + p1 * 8],
                num_idxs=ni,
                num_idxs_reg=cnt_reg,
                elem_size=D,
                queue_num=qrr[0] % NQ,
            )
            qrr[0] += 1

    feat_h = nc.dram_tensor("features_h", [N, D], f16, kind="ExternalInput")
    edge_idx_d = nc.dram_tensor(
        "edge_idx", [P, IDX_COLS], mybir.dt.int16, kind="ExternalInput"
    )
    seg_d = nc.dram_tensor("seg", [P, T_TOTAL], f16, kind="ExternalInput")
    invdeg_d = nc.dram_tensor(
        "invdeg", [P, NBLK_pad * P], f32, kind="ExternalInput"
    )
    w1t_d = nc.dram_tensor("w1t", [D, D], f32, kind="ExternalInput")
    w2t_d = nc.dram_tensor("w2t", [D, D], f32, kind="ExternalInput")
    bias_d = nc.dram_tensor("bias_col", [D, P], f32, kind="ExternalInput")
    iota_d = nc.dram_tensor("iota", [P, 256], f16, kind="ExternalInput")
    out_d = nc.dram_tensor(
        "out", [D, NBLK_pad * P], f32, kind="ExternalOutput"
    )

    with TileContext(nc) as tc:
        with (
            tc.tile_pool(name="const", bufs=1) as const_pool,
            tc.tile_pool(name="ge", bufs=ge_bufs) as ge_pool,
            tc.tile_pool(name="s", bufs=s_bufs) as s_pool,
            tc.tile_pool(name="blk", bufs=blk_bufs) as blk_pool,
            tc.tile_pool(name="psum1", bufs=p1_bufs, space="PSUM") as psum1_pool,
            tc.tile_pool(name="psum2", bufs=p2_bufs, space="PSUM") as psum2_pool,
        ):
            nc.gpsimd.load_library(mlp)

            def load_const(dram, shape, dtype=f32, tag=None):
                t = const_pool.tile(shape, dtype, tag=tag)
                nc.sync.dma_start(t[:], dram[:])
                return t

            edge_idx = load_const(
                edge_idx_d, [P, IDX_COLS], mybir.dt.int16, tag="edge_idx"
            )
            seg = load_const(seg_d, [P, T_TOTAL], f16, tag="seg")
            invdeg = load_const(
                invdeg_d, [P, NBLK_pad * P], tag="invdeg"
            ).rearrange("p (b s) -> p b s", s=P)
            w1t = load_const(w1t_d, [D, D], tag="w1t")
            w2t = load_const(w2t_d, [D, D], tag="w2t")
            bias_col = load_const(bias_d, [D, P], tag="bias_col")
            iota = load_const(iota_d, [P, 256], f16, tag="iota")

            for _rep in range(reps):
                for g in range(NBG):
                    gbuf = {}
                    stile = {}
                    for k in range(nchunk):
                        tgk = int(EG_CAP[g, k]) // P
                        if tgk == 0:
                            continue
                        gb = ge_pool.tile([P, EG_TILES_MAX, D], f16, tag="ge")
                        emit_gather(
                            gb, 0, tgk,
                            feat_h[plan["win_base"][k] :, :],
                            edge_idx, int(idx_off[g, k]),
                        )
                        gbuf[k] = gb
                        # batched one-hot: S[p, t, w] = (seg[p, col0+t] == iota[w])
                        # w in [0, 256): left half neighbor edges, right half self
                        st = s_pool.tile([P, EG_TILES_MAX, 256], f16, tag="s")
                        c0 = int(col0[g, k])
                        seg_rep = seg[:, c0 : c0 + tgk].rearrange(
                            "p (t o) -> p t o", o=1
                        ).to_broadcast([P, tgk, 256])
                        iota_rep = iota[:, :].rearrange(
                            "p (o w) -> p o w", o=1
                        ).to_broadcast([P, tgk, 256])
                        nc.vector.tensor_tensor(
                            out=st[:, :tgk, :],
                            in0=seg_rep,
                            in1=iota_rep,
                            op=mybir.AluOpType.is_equal,
                        )
                        stile[k] = st

                    psum1 = psum1_pool.tile([P, nbg_blocks, 256], f32, tag="p1")
                    # per-block static schedule of (chunk, local tile) pairs
                    sched = [[] for _ in range(nbg_blocks)]
                    for k in range(nchunk):
                        off = 0
                        for bb in range(nbg_blocks):
                            tcount = int(T[g * nbg_blocks + bb, k])
                            for t in range(tcount):
                                sched[bb].append((k, off + t))
                            off += tcount
                    # block-major: one PSUM accumulation group open at a time
                    for bb in range(nbg_blocks):
                        total = len(sched[bb])
                        for i, (k, t) in enumerate(sched[bb]):
                            nc.tensor.matmul(
                                out=psum1[:, bb, :],
                                lhsT=gbuf[k][:, t, :],
                                rhs=stile[k][:, t, :],
                                start=(i == 0),
                                stop=(i == total - 1),
                            )

                    for bb in range(nbg_blocks):
                        blk = g * nbg_blocks + bb
                        if blk in plan["skip_blocks"]:
                            continue
                        # neighbor mean: PSUM left half * invdeg -> SBUF
                        msum = blk_pool.tile([P, P], f32, tag="msum")
                        nc.vector.tensor_tensor(
                            out=msum[:],
                            in0=psum1[:, bb, 0:128],
                            in1=invdeg[:, blk, :],
                            op=mybir.AluOpType.mult,
                        )
                        # self features: PSUM right half -> SBUF
                        selfT = blk_pool.tile([P, P], f32, tag="selfT")
                        nc.scalar.activation(
                            selfT[:],
                            psum1[:, bb, 128:256],
                            mybir.ActivationFunctionType.Copy,
                        )
                        psum_o = psum2_pool.tile([P, P], f32, tag="po")
                        nc.tensor.matmul(
                            out=psum_o[:], lhsT=w2t[:], rhs=msum[:],
                            start=True, stop=False,
                        )
                        nc.tensor.matmul(
                            out=psum_o[:], lhsT=w1t[:], rhs=selfT[:],
                            start=False, stop=True,
                        )
                        out_sb = blk_pool.tile([P, P], f32, tag="osb")
                        if plan["bias_nonzero"]:
                            o1 = blk_pool.tile([P, P], f32, tag="o1")
                            nc.vector.tensor_tensor(
                                out=o1[:], in0=psum_o[:], in1=bias_col[:],
                                op=mybir.AluOpType.add,
                            )
                            nc.scalar.activation(
                                out_sb[:], o1[:], mybir.ActivationFunctionType.Relu
                            )
                        else:
                            nc.scalar.activation(
                                out_sb[:], psum_o[:],
                                mybir.ActivationFunctionType.Relu,
                            )
                        nc.sync.dma_start(
                            out_d[:, blk * P : (blk + 1) * P], out_sb[:]
                        )

    nc.compile()
    return nc


_RUN_KWARGS = {}


def run_on_hw(nc, in_maps, n_cores, **kwargs):
    from concourse.bass_utils import run_bass_kernel_spmd

    return run_bass_kernel_spmd(nc, in_maps, list(range(n_cores)), **kwargs)


def kernel(nodes, features, edge_index, W, b):
    """Full-input entry point: shards internally across 8 NeuronCores."""
    n_cores = 8
    plan, in_maps, assemble = preprocess(
        nodes, features, edge_index, W, b, n_cores=n_cores
    )
    nc = build_kernel(plan)
    res = run_on_hw(nc, in_maps, n_cores, **_RUN_KWARGS)
    outs = [np.asarray(r["out"]) for r in res.results]
    return np.ascontiguousarray(assemble(outs).astype(np.float32))
